# revision 1
# baseline (speedup 1.0000x reference)
"""Discriminative loss on 8 Trainium2 NeuronCores.

Strategy (data-parallel over batch: one sample per core):
  Inputs per sample: prediction p (16, L=262144) f32, target g (16, L) one-hot f32.
  Folded on-chip layout: (16, 8*32768) -> (128, 32768) fp16 (contiguous reshape),
  partition index m = c*16 + x (x = dim d for p / instance i for g, c = column chunk).

  Phase A (streaming, one HBM pass): gpsimd casting DMAs load p,g as fp16 into
  SBUF-resident tiles. Per 128-column chunk, PE transposes p and g chunks
  (pixel-major), then one accumulating matmul builds the gram
  gram[(i,c), (d,c') | cnt] = sum_pix gT . [pT | 1], whose c==c' diagonal blocks
  are the cluster-mean numerators and whose last column is per-(i,c) pixel counts.

  Epilogue (on device, tiny): means = num/clip(cnt,1); m2 = |mu|^2; weight matrix
  meansBD = block-diag(-2*mu^T) fp16; bias b = m2 - M (M = mask offset).

  Phase B (SBUF-resident, no HBM): per (128,512) tile, three accumulating matmuls
  produce psum = -2*pm + p2 + M*g for all 16 instances of every pixel; then
  sq = max(psum + (m2 - M), 0) (invalid/other-instance entries forced negative ->
  0 -> hinge 0), dist = sqrt(sq), h = max(dist - 0.5, 0), and ACT Square
  accumulates sum(h^2) per (i,c) partition.

  Host: combines per-core [acc (128), cnt (128), means (16,16)] in float64:
  var term from acc/cnt, pairwise-distance + regularizer terms from means,
  mean over batch. No collectives needed.
"""

import sys
import os
import numpy as np

for _p in ("/opt/trn_rl_repo", "/root/.axon_site/_ro/pypackages"):
    if os.path.isdir(_p) and _p not in sys.path:
        sys.path.insert(0, _p)

BS, ND, H, W, NI = 8, 16, 512, 512, 16
L = H * W                  # 262144 pixels per sample
C = 8                      # fold factor (partition = x*8 + c)
R = L // C                 # 32768 folded free dim
NBT = 16                   # big DMA tiles per tensor
TB = R // NBT              # 2048 cols per big tile
NCH = R // 128             # 256 transpose chunks
NT = 64                    # phase-B compute tiles
TF = R // NT               # 512 cols per compute tile
N_CORES = 8
MBIG = 1024.0              # mask offset, exact in fp16, >> max ||p - mu||^2
DELTA_VAR = 0.5
DELTA_DIST = 1.5
VAR_W, DIST_W, REG_W = 1.0, 1.0, 0.001
EPS = 1e-12

_CACHE = {}


def _host_consts():
    f16 = np.float16
    ident = np.eye(128, dtype=f16)
    # onesbd[(c',d),(c,i)] = 1 if c == c'  (p2 broadcast over instances)
    onesbd = np.zeros((128, 128), dtype=f16)
    for c in range(C):
        onesbd[16 * c:16 * c + 16, 16 * c:16 * c + 16] = 1.0
    mibig = (MBIG * np.eye(128)).astype(f16)
    # repmat[k, c*16+i] = (k == i): replicates (16,1) m2 to (128,1) over c
    repmat = np.zeros((16, 128), dtype=np.float32)
    for i in range(16):
        repmat[i, i::16] = 1.0
    ident16 = np.eye(16, dtype=np.float32)
    return {
        "ident": ident,
        "onesbd": onesbd,
        "mibig": mibig,
        "repmat": repmat,
        "ident16": ident16,
    }


def _build(reps=1):
    import concourse.bass as bass
    import concourse.tile as tile
    from concourse import bacc, mybir

    f32 = mybir.dt.float32
    f16 = mybir.dt.float16
    Alu = mybir.AluOpType
    Act = mybir.ActivationFunctionType

    nc = bacc.Bacc("TRN2", target_bir_lowering=False, debug=False,
                   num_devices=N_CORES)

    p_dram = nc.dram_tensor("p", [16, L], f32, kind="ExternalInput").ap()
    g_dram = nc.dram_tensor("g", [16, L], f32, kind="ExternalInput").ap()
    ident_d = nc.dram_tensor("ident", [128, 128], f16, kind="ExternalInput").ap()
    onesbd_d = nc.dram_tensor("onesbd", [128, 128], f16, kind="ExternalInput").ap()
    mibig_d = nc.dram_tensor("mibig", [128, 128], f16, kind="ExternalInput").ap()
    repmat_d = nc.dram_tensor("repmat", [16, 128], f32, kind="ExternalInput").ap()
    ident16_d = nc.dram_tensor("ident16", [16, 16], f32, kind="ExternalInput").ap()

    out_acc = nc.dram_tensor("out_acc", [128], f32, kind="ExternalOutput").ap()
    out_cnt = nc.dram_tensor("out_cnt", [128], f32, kind="ExternalOutput").ap()
    out_means = nc.dram_tensor("out_means", [16, 16], f32, kind="ExternalOutput").ap()

    # c-major folded view (c, d, r): DMA iteration order (c,d) maps to the
    # 128 SBUF partitions as m = c*16 + d
    p_fold = p_dram.rearrange("d (c r) -> d c r", c=C).transpose((1, 0, 2))
    g_fold = g_dram.rearrange("d (c r) -> d c r", c=C).transpose((1, 0, 2))

    with tile.TileContext(nc, num_cores=N_CORES) as tc:
        from contextlib import ExitStack, nullcontext
        with ExitStack() as ctx:
            const_pool = ctx.enter_context(tc.tile_pool(name="const", bufs=1))
            ident = const_pool.tile([128, 128], f16, tag="ident")
            nc.sync.dma_start(ident, ident_d)
            onesbd = const_pool.tile([128, 128], f16, tag="onesbd")
            nc.sync.dma_start(onesbd, onesbd_d)
            mibig = const_pool.tile([128, 128], f16, tag="mibig")
            nc.sync.dma_start(mibig, mibig_d)
            repmat = const_pool.tile([16, 128], f32, tag="repmat")
            nc.sync.dma_start(repmat, repmat_d)
            ident16 = const_pool.tile([16, 16], f32, tag="ident16")
            nc.sync.dma_start(ident16, ident16_d)

            p_pool = ctx.enter_context(tc.tile_pool(name="p16", bufs=NBT))
            g_pool = ctx.enter_context(tc.tile_pool(name="g16", bufs=NBT))
            small = ctx.enter_context(tc.tile_pool(name="small", bufs=1))
            gram_pool = ctx.enter_context(
                tc.tile_pool(name="gram", bufs=1, space="PSUM"))
            psA_p = ctx.enter_context(
                tc.tile_pool(name="psA_p", bufs=2, space="PSUM"))
            psA_g = ctx.enter_context(
                tc.tile_pool(name="psA_g", bufs=2, space="PSUM"))
            psB = ctx.enter_context(
                tc.tile_pool(name="psB", bufs=3, space="PSUM"))
            aT = ctx.enter_context(tc.tile_pool(name="aT", bufs=6))
            wB = ctx.enter_context(tc.tile_pool(name="wB", bufs=6))

            loop = tc.For_i(0, reps, 1) if reps > 1 else nullcontext()
            with loop:
                # resident fp16 copies of p and g (8 MiB each), cast on DMA
                p16 = []
                g16 = []
                for j in range(NBT):
                    pt = p_pool.tile([128, TB], f16, tag=f"p{j}", bufs=1)
                    nc.gpsimd.dma_start(pt, p_fold[:, :, j * TB:(j + 1) * TB])
                    gt = g_pool.tile([128, TB], f16, tag=f"g{j}", bufs=1)
                    nc.gpsimd.dma_start(gt, g_fold[:, :, j * TB:(j + 1) * TB])
                    p16.append(pt)
                    g16.append(gt)

                acc_cols = small.tile([128, NT], f32, tag="acc_cols")
                cntA = small.tile([128, NBT], f32, tag="cntA")
                negM = small.tile([128, 1], f32, tag="negM")
                nc.vector.memset(negM, -MBIG)

                # ------------- Phase A: gram + counts -------------
                gram = gram_pool.tile([128, 132], f32, tag="gram")
                for j in range(NBT):
                    # per-(c,i) pixel counts: free-dim reduction of one-hot g
                    nc.vector.reduce_sum(cntA[:, j:j + 1], g16[j],
                                         axis=mybir.AxisListType.X)
                TPB = TB // TF           # phase-B tiles per big tile
                sqm = []
                for j in range(NBT):
                    for kk in range(TB // 128):
                        k = j * (TB // 128) + kk
                        off = kk * 128
                        pch = p16[j][:, off:off + 128]
                        gch = g16[j][:, off:off + 128]
                        tp = psA_p.tile([128, 128], f16, tag="tp")
                        nc.tensor.transpose(tp, pch, ident)
                        tg = psA_g.tile([128, 128], f16, tag="tg")
                        nc.tensor.transpose(tg, gch, ident)
                        rhs = aT.tile([128, 128], f16, tag="rhs")
                        nc.vector.tensor_copy(rhs, tp)
                        gT = aT.tile([128, 128], f16, tag="gT")
                        nc.scalar.copy(gT, tg)
                        nc.tensor.matmul(gram[:, 0:128], lhsT=gT, rhs=rhs,
                                         start=(k == 0), stop=(k == NCH - 1))
                    # mean-independent part of the distance computation:
                    # psum2 = p2 + M*g per (c,i), evacuated as fp16 partials
                    # into g16[j]'s SBUF slot (g dies after mm3 here)
                    sqtmp = []
                    for tt in range(TPB):
                        off = tt * TF
                        pch = p16[j][:, off:off + TF]
                        gch = g16[j][:, off:off + TF]
                        psq = wB.tile([128, TF], f16, tag="psq")
                        nc.vector.tensor_tensor(psq, pch, pch, op=Alu.mult)
                        sqp2 = psB.tile([128, TF], f32, tag="sqp2")
                        nc.tensor.matmul(sqp2, lhsT=onesbd, rhs=psq,
                                         start=True, stop=False)
                        nc.tensor.matmul(sqp2, lhsT=mibig, rhs=gch,
                                         start=False, stop=True)
                        # fp16(p2 + M*g - M): own-instance entries keep p2
                        # precisely; others go very negative (clipped later)
                        st = wB.tile([128, TF], f16, tag="sqt")
                        nc.scalar.activation(st, sqp2, Act.Identity, bias=negM)
                        sqtmp.append(st)
                    sqm_j = g_pool.tile([128, TB], f16, tag=f"g{j}", bufs=1)
                    for tt in range(TPB):
                        nc.vector.tensor_copy(
                            sqm_j[:, tt * TF:(tt + 1) * TF], sqtmp[tt])
                    sqm.append(sqm_j)

                # ---------- epilogue: means, m2, weights ----------
                gram_sb = small.tile([128, 132], f32, tag="gram_sb")
                nc.vector.tensor_copy(gram_sb[:, 0:128], gram[:, 0:128])

                # gather the 8 diagonal (i,d) blocks + count slices into
                # partition-aligned tiles (DMA remaps partitions; vector ops
                # cannot read across partition lanes)
                blocks = small.tile([16, C * 16], f32, tag="blocks")
                cnt8 = small.tile([16, C], f32, tag="cnt8")
                cnt_sb = small.tile([128, 1], f32, tag="cnt_sb")
                nc.vector.reduce_sum(cnt_sb, cntA, axis=mybir.AxisListType.X)
                for c in range(C):
                    nc.sync.dma_start(blocks[:, c * 16:(c + 1) * 16],
                                      gram_sb[16 * c:16 * c + 16,
                                              16 * c:16 * c + 16])
                    nc.sync.dma_start(cnt8[:, c:c + 1],
                                      cnt_sb[16 * c:16 * c + 16, :])

                mnum = small.tile([16, 16], f32, tag="mnum")
                nc.vector.reduce_sum(
                    mnum, blocks.rearrange("i (c d) -> i d c", c=C),
                    axis=mybir.AxisListType.X)
                gsum = small.tile([16, 1], f32, tag="gsum")
                nc.vector.reduce_sum(gsum, cnt8, axis=mybir.AxisListType.X)
                gsum_c = small.tile([16, 1], f32, tag="gsum_c")
                nc.vector.tensor_scalar(gsum_c, gsum, 1.0, None, op0=Alu.max)
                invg = small.tile([16, 1], f32, tag="invg")
                nc.vector.reciprocal(invg, gsum_c)

                means = small.tile([16, 16], f32, tag="means")
                nc.vector.tensor_scalar(means, mnum, invg, None, op0=Alu.mult)
                nc.sync.dma_start(out_means, means)
                nc.sync.dma_start(out_cnt, cnt_sb)

                # m2 and bias b = m2 - M, replicated per (c,i) partition
                msq = small.tile([16, 16], f32, tag="msq")
                nc.vector.tensor_tensor(msq, means, means, op=Alu.mult)
                m2 = small.tile([16, 1], f32, tag="m2")
                nc.vector.reduce_sum(m2, msq, axis=mybir.AxisListType.X)
                m2p = gram_pool.tile([128, 1], f32, tag="gram")
                nc.tensor.matmul(m2p, lhsT=repmat, rhs=m2,
                                 start=True, stop=True)
                b_part = small.tile([128, 1], f32, tag="b_part")
                nc.vector.tensor_scalar(b_part, m2p, 0.0, None, op0=Alu.add)
                # meansBD[(c,d),(c,i)] = -2 * means[i,d] (fp16 block diag)
                meansT = gram_pool.tile([16, 16], f32, tag="gram")
                nc.tensor.transpose(meansT, means, ident16)
                mT2 = small.tile([16, 16], f16, tag="mT2")
                nc.scalar.mul(mT2, meansT, -2.0)
                meansBD = small.tile([128, 128], f16, tag="meansBD")
                nc.vector.memset(meansBD, 0.0)
                for c in range(C):
                    nc.sync.dma_start(
                        meansBD[16 * c:16 * c + 16, 16 * c:16 * c + 16], mT2)

                # ------------- Phase B: per-pixel distances -------------
                for t in range(NT):
                    j, off = divmod(t * TF, TB)
                    pch = p16[j][:, off:off + TF]
                    sqm_ch = sqm[j][:, off:off + TF]
                    sqp = psB.tile([128, TF], f32, tag="sqp2")
                    nc.tensor.matmul(sqp, lhsT=meansBD, rhs=pch,
                                     start=True, stop=True)
                    sq16 = wB.tile([128, TF], f16, tag="sq")
                    nc.vector.scalar_tensor_tensor(
                        sq16, sqp, b_part, sqm_ch, op0=Alu.add, op1=Alu.add)
                    sqc = wB.tile([128, TF], f16, tag="sqc")
                    if t % 2 == 0:
                        nc.vector.tensor_scalar(sqc, sq16, 0.0, None,
                                                op0=Alu.max)
                    else:
                        nc.scalar.activation(sqc, sq16, Act.Relu)
                    dist = wB.tile([128, TF], f16, tag="dist")
                    nc.scalar.activation(dist, sqc, Act.Sqrt)
                    h = wB.tile([128, TF], f16, tag="h")
                    nc.gpsimd.tensor_scalar(h, dist, DELTA_VAR, 0.0,
                                            op0=Alu.subtract, op1=Alu.max)
                    h2d = wB.tile([128, TF], f16, tag="h2")
                    nc.scalar.activation(h2d, h, Act.Square,
                                         accum_out=acc_cols[:, t:t + 1])

                var_col = small.tile([128, 1], f32, tag="var_col")
                nc.vector.reduce_sum(var_col, acc_cols,
                                     axis=mybir.AxisListType.X)
                nc.sync.dma_start(out_acc, var_col)

    nc.compile()
    return nc


def _get_nc(reps=1):
    key = ("nc", reps)
    if key not in _CACHE:
        _CACHE[key] = _build(reps)
    return _CACHE[key]


def _host_combine(accs, cnts, means_all, n_objects):
    """Per-core device outputs -> final scalar loss (float64 on host)."""
    losses = []
    for b in range(BS):
        no = float(n_objects[b])
        acc = accs[b].astype(np.float64).reshape(C, 16).sum(axis=0)
        cnt = cnts[b].astype(np.float64).reshape(C, 16).sum(axis=0)
        means = means_all[b].astype(np.float64)            # (i, d)
        valid = (np.arange(NI) < n_objects[b]).astype(np.float64)

        g_sum = np.clip(cnt, 1.0, None)
        var_term = float(np.sum(acc / g_sum) / no)

        means_m = means * valid[:, None]
        diff = means_m[:, None, :] - means_m[None, :, :]
        psq = np.clip((diff * diff).sum(-1), EPS, None)
        pnorm = np.sqrt(psq)
        eye = np.eye(NI)
        margin = 2.0 * DELTA_DIST * (1.0 - eye)
        pair_mask = valid[:, None] * valid[None, :] * (1.0 - eye)
        hinge = np.clip(margin - pnorm, 0.0, None) ** 2 * pair_mask
        denom = max(no * (no - 1.0), 1.0)
        multi = 1.0 if n_objects[b] > 1 else 0.0
        dist_term = float(hinge.sum() / denom * multi)

        mnorm = np.sqrt(np.clip((means_m * means_m).sum(-1), EPS, None)) * valid
        reg_term = float(mnorm.sum() / no)

        losses.append(VAR_W * var_term + DIST_W * dist_term + REG_W * reg_term)
    return np.float32(np.mean(losses))


def _run(prediction, target, n_objects, trace=False, reps=1, **spmd_kwargs):
    from concourse.bass_utils import run_bass_kernel_spmd

    nc = _get_nc(reps)
    consts = _host_consts()

    pred = np.ascontiguousarray(np.asarray(prediction, dtype=np.float32))
    targ = np.ascontiguousarray(np.asarray(target, dtype=np.float32))
    nobj = np.asarray(n_objects)

    in_maps = []
    for b in range(BS):
        m = {"p": pred[b].reshape(16, L), "g": targ[b].reshape(16, L)}
        m.update(consts)
        in_maps.append(m)

    res = run_bass_kernel_spmd(nc, in_maps, list(range(N_CORES)),
                               trace=trace, **spmd_kwargs)
    accs = [res.results[b]["out_acc"] for b in range(BS)]
    cnts = [res.results[b]["out_cnt"] for b in range(BS)]
    means = [res.results[b]["out_means"] for b in range(BS)]
    return _host_combine(accs, cnts, means, nobj), res


def kernel(prediction, target, n_objects):
    loss, _ = _run(prediction, target, n_objects)
    return loss



# revision 2
# speedup vs baseline: 3.1895x; 3.1895x over previous
"""Discriminative loss on 8 Trainium2 NeuronCores — v2.

Strategy (data-parallel over batch: one sample per core):
  Same dataflow as v1 (fold (16, L) -> (128, L/8) fp16, PE-transpose gram
  for cluster means, masked per-pixel distance phase), rebalanced so every
  engine stays under the ~93us HBM stream and phase B's critical path is
  short:

  Phase A (streaming, under the casting-DMA shadow):
    - gpsimd casting DMAs load p,g fp16 (32 MiB f32 read per core).
    - counts via DVE reduce; psq = p*p via DVE (4x fp16).
    - PE transposes p/g chunks into grouped [128,512] f16 PSUM tiles;
      Act evacuates p-groups, gpsimd evacuates g-groups; PE accumulates
      the gram.
    - sqm = fp16(p^2 + M*g - M) via PE (onesbd/mibig matmuls) + DVE
      evacuation, parked then copied into g's SBUF (g dies after this).

  Epilogue: means from gram diag blocks; meansBD = block-diag(-2 mu^T)
  fp16; b_part = m2 (per-partition bias).

  Phase B per (128,512) tile: psum = meansBD@p + ident@sqm (2 accum
  matmuls); dist = Act.Sqrt(psum + b_part) — masked entries go negative
  -> NaN; h = max(dist - 0.5, 0) on DVE/gpsimd KILLS NaN (HW-verified);
  h2 = h*h and per-tile accumulation on DVE.

  Host: combines per-core [acc, cnt, means] in float64 (var term, pairwise
  distance + regularizer from means), mean over batch. No collectives.
"""

import sys
import os
import numpy as np

for _p in ("/opt/trn_rl_repo", "/root/.axon_site/_ro/pypackages"):
    if os.path.isdir(_p) and _p not in sys.path:
        sys.path.insert(0, _p)

BS, ND, H, W, NI = 8, 16, 512, 512, 16
L = H * W                  # 262144 pixels per sample
C = 8                      # fold factor (partition = c*16 + x)
R = L // C                 # 32768 folded free dim
NBT = 16                   # big DMA tiles per tensor
TB = R // NBT              # 2048 cols per big tile
NCH = R // 128             # 256 transpose chunks
NT = 64                    # phase-B compute tiles
TF = R // NT               # 512 cols per compute tile
N_CORES = 8
MBIG = 1024.0              # mask offset, exact in fp16
DELTA_VAR = 0.5
DELTA_DIST = 1.5
VAR_W, DIST_W, REG_W = 1.0, 1.0, 0.001
EPS = 1e-12

_CACHE = {}


def _host_consts():
    f16 = np.float16
    ident = np.eye(128, dtype=f16)
    # onesbd[(c',d),(c,i)] = 1 if c == c'  (p2 broadcast over instances)
    onesbd = np.zeros((128, 128), dtype=f16)
    for c in range(C):
        onesbd[16 * c:16 * c + 16, 16 * c:16 * c + 16] = 1.0
    mibig = (MBIG * np.eye(128)).astype(f16)
    # repmat[k, c*16+i] = (k == i): replicates (16,1) m2 to (128,1) over c
    repmat = np.zeros((16, 128), dtype=np.float32)
    for i in range(16):
        repmat[i, i::16] = 1.0
    ident16 = np.eye(16, dtype=np.float32)
    return {
        "ident": ident,
        "onesbd": onesbd,
        "mibig": mibig,
        "repmat": repmat,
        "ident16": ident16,
    }


def _build(reps=1):
    import concourse.bass as bass
    import concourse.tile as tile
    from concourse import bacc, mybir

    f32 = mybir.dt.float32
    f16 = mybir.dt.float16
    Alu = mybir.AluOpType
    Act = mybir.ActivationFunctionType

    nc = bacc.Bacc("TRN2", target_bir_lowering=False, debug=False,
                   num_devices=N_CORES)

    p_dram = nc.dram_tensor("p", [16, L], f32, kind="ExternalInput").ap()
    g_dram = nc.dram_tensor("g", [16, L], f32, kind="ExternalInput").ap()
    ident_d = nc.dram_tensor("ident", [128, 128], f16, kind="ExternalInput").ap()
    onesbd_d = nc.dram_tensor("onesbd", [128, 128], f16, kind="ExternalInput").ap()
    mibig_d = nc.dram_tensor("mibig", [128, 128], f16, kind="ExternalInput").ap()
    repmat_d = nc.dram_tensor("repmat", [16, 128], f32, kind="ExternalInput").ap()
    ident16_d = nc.dram_tensor("ident16", [16, 16], f32, kind="ExternalInput").ap()

    out_acc = nc.dram_tensor("out_acc", [128], f32, kind="ExternalOutput").ap()
    out_cnt = nc.dram_tensor("out_cnt", [128], f32, kind="ExternalOutput").ap()
    out_means = nc.dram_tensor("out_means", [16, 16], f32, kind="ExternalOutput").ap()

    p_fold = p_dram.rearrange("d (c r) -> d c r", c=C).transpose((1, 0, 2))
    g_fold = g_dram.rearrange("d (c r) -> d c r", c=C).transpose((1, 0, 2))

    GRP = 4                  # transpose chunks per PSUM group
    NGR = (TB // 128) // GRP  # groups per big tile (4)

    with tile.TileContext(nc, num_cores=N_CORES) as tc:
        from contextlib import ExitStack, nullcontext
        with ExitStack() as ctx:
            const_pool = ctx.enter_context(tc.tile_pool(name="const", bufs=1))
            ident = const_pool.tile([128, 128], f16, tag="ident")
            nc.sync.dma_start(ident, ident_d)
            onesbd = const_pool.tile([128, 128], f16, tag="onesbd")
            nc.sync.dma_start(onesbd, onesbd_d)
            mibig = const_pool.tile([128, 128], f16, tag="mibig")
            nc.sync.dma_start(mibig, mibig_d)
            repmat = const_pool.tile([16, 128], f32, tag="repmat")
            nc.sync.dma_start(repmat, repmat_d)
            ident16 = const_pool.tile([16, 16], f32, tag="ident16")
            nc.sync.dma_start(ident16, ident16_d)

            p_pool = ctx.enter_context(tc.tile_pool(name="p16", bufs=NBT))
            g_pool = ctx.enter_context(tc.tile_pool(name="g16", bufs=NBT))
            small = ctx.enter_context(tc.tile_pool(name="small", bufs=1))
            gram_pool = ctx.enter_context(
                tc.tile_pool(name="gram", bufs=1, space="PSUM"))
            psT_p = ctx.enter_context(
                tc.tile_pool(name="psT_p", bufs=2, space="PSUM"))
            psT_g = ctx.enter_context(
                tc.tile_pool(name="psT_g", bufs=2, space="PSUM"))
            psSQ = ctx.enter_context(
                tc.tile_pool(name="psSQ", bufs=2, space="PSUM"))
            psB = psSQ
            aT = ctx.enter_context(tc.tile_pool(name="aT", bufs=6))
            wB = ctx.enter_context(tc.tile_pool(name="wB", bufs=8))

            loop = tc.For_i(0, reps, 1) if reps > 1 else nullcontext()
            with loop:
                # resident fp16 copies of p and g, cast on DMA
                p16 = []
                g16 = []
                for j in range(NBT):
                    pt = p_pool.tile([128, TB], f16, tag=f"p{j}", bufs=1)
                    nc.gpsimd.dma_start(pt, p_fold[:, :, j * TB:(j + 1) * TB])
                    gt = g_pool.tile([128, TB], f16, tag=f"g{j}", bufs=1)
                    nc.gpsimd.dma_start(gt, g_fold[:, :, j * TB:(j + 1) * TB])
                    p16.append(pt)
                    g16.append(gt)

                acc_cols = small.tile([128, NT], f32, tag="acc_cols")
                cntA = small.tile([128, NBT], f32, tag="cntA")

                # ------------- Phase A: gram + counts + sqm -------------
                gram = gram_pool.tile([128, 128], f32, tag="gram")
                sqm = []
                for j in range(NBT):
                    nc.vector.reduce_sum(cntA[:, j:j + 1], g16[j],
                                         axis=mybir.AxisListType.X)
                    # transposes (grouped 4 chunks per PSUM tile) + gram
                    for q in range(NGR):
                        tpg = psT_p.tile([128, GRP * 128], f16, tag="tpg")
                        tgg = psT_g.tile([128, GRP * 128], f16, tag="tgg")
                        for k in range(GRP):
                            off = (q * GRP + k) * 128
                            nc.tensor.transpose(
                                tpg[:, k * 128:(k + 1) * 128],
                                p16[j][:, off:off + 128], ident)
                            nc.tensor.transpose(
                                tgg[:, k * 128:(k + 1) * 128],
                                g16[j][:, off:off + 128], ident)
                        rhs = aT.tile([128, GRP * 128], f16, tag="rhs")
                        nc.scalar.copy(rhs, tpg)
                        gT = aT.tile([128, GRP * 128], f16, tag="gT")
                        nc.scalar.copy(gT, tgg)
                        for k in range(GRP):
                            kk = j * (TB // 128) + q * GRP + k
                            nc.tensor.matmul(
                                gram, lhsT=gT[:, k * 128:(k + 1) * 128],
                                rhs=rhs[:, k * 128:(k + 1) * 128],
                                start=(kk == 0), stop=(kk == NCH - 1))
                    # sqm = fp16(p^2 + M*g - M), parked then copied into
                    # g16[j]'s SBUF slot (g dies after the mibig matmul)
                    TPB = TB // TF
                    park = []
                    for tt in range(TPB):
                        off = tt * TF
                        pch = p16[j][:, off:off + TF]
                        gch = g16[j][:, off:off + TF]
                        psq = wB.tile([128, TF], f16, tag="psq")
                        nc.vector.tensor_tensor(psq, pch, pch, op=Alu.mult)
                        sq2 = psSQ.tile([128, TF], f32, tag="sq2")
                        nc.tensor.matmul(sq2, lhsT=onesbd, rhs=psq,
                                         start=True, stop=False)
                        nc.tensor.matmul(sq2, lhsT=mibig, rhs=gch,
                                         start=False, stop=True)
                        st = wB.tile([128, TF], f16, tag="sqt")
                        nc.vector.tensor_scalar(st, sq2, -MBIG, None,
                                                op0=Alu.add)
                        park.append(st)
                    sqm_j = g_pool.tile([128, TB], f16, tag=f"g{j}", bufs=1)
                    for tt in range(TPB):
                        nc.gpsimd.tensor_copy(
                            sqm_j[:, tt * TF:(tt + 1) * TF], park[tt])
                    sqm.append(sqm_j)

                # ---------- epilogue: means, m2, weights ----------
                gram_sb = small.tile([128, 128], f32, tag="gram_sb")
                nc.vector.tensor_copy(gram_sb, gram)

                blocks = small.tile([16, C * 16], f32, tag="blocks")
                cnt8 = small.tile([16, C], f32, tag="cnt8")
                cnt_sb = small.tile([128, 1], f32, tag="cnt_sb")
                nc.vector.reduce_sum(cnt_sb, cntA, axis=mybir.AxisListType.X)
                for c in range(C):
                    nc.sync.dma_start(blocks[:, c * 16:(c + 1) * 16],
                                      gram_sb[16 * c:16 * c + 16,
                                              16 * c:16 * c + 16])
                    nc.scalar.dma_start(cnt8[:, c:c + 1],
                                        cnt_sb[16 * c:16 * c + 16, :])

                mnum = small.tile([16, 16], f32, tag="mnum")
                nc.vector.reduce_sum(
                    mnum, blocks.rearrange("i (c d) -> i d c", c=C),
                    axis=mybir.AxisListType.X)
                gsum = small.tile([16, 1], f32, tag="gsum")
                nc.vector.reduce_sum(gsum, cnt8, axis=mybir.AxisListType.X)
                gsum_c = small.tile([16, 1], f32, tag="gsum_c")
                nc.vector.tensor_scalar(gsum_c, gsum, 1.0, None, op0=Alu.max)
                invg = small.tile([16, 1], f32, tag="invg")
                nc.vector.reciprocal(invg, gsum_c)

                means = small.tile([16, 16], f32, tag="means")
                nc.vector.tensor_scalar(means, mnum, invg, None, op0=Alu.mult)
                nc.sync.dma_start(out_means, means)
                nc.sync.dma_start(out_cnt, cnt_sb)

                # m2 replicated per (c,i) partition -> bias b = m2
                msq = small.tile([16, 16], f32, tag="msq")
                nc.vector.tensor_tensor(msq, means, means, op=Alu.mult)
                m2 = small.tile([16, 1], f32, tag="m2")
                nc.vector.reduce_sum(m2, msq, axis=mybir.AxisListType.X)
                m2p = psSQ.tile([128, 1], f32, tag="sq2")
                nc.tensor.matmul(m2p, lhsT=repmat, rhs=m2,
                                 start=True, stop=True)
                b_part = small.tile([128, 1], f32, tag="b_part")
                nc.vector.tensor_copy(b_part, m2p)
                # meansBD[(c,d),(c,i)] = -2 * means[i,d] (fp16 block diag)
                meansT = psSQ.tile([16, 16], f32, tag="sq2")
                nc.tensor.transpose(meansT, means, ident16)
                mT2 = small.tile([16, 16], f16, tag="mT2")
                nc.scalar.mul(mT2, meansT, -2.0)
                meansBD = small.tile([128, 128], f16, tag="meansBD")
                nc.vector.memset(meansBD, 0.0)
                for c in range(C):
                    nc.sync.dma_start(
                        meansBD[16 * c:16 * c + 16, 16 * c:16 * c + 16], mT2)

                # ------------- Phase B: per-pixel distances -------------
                for t in range(NT):
                    j, off = divmod(t * TF, TB)
                    pch = p16[j][:, off:off + TF]
                    sqm_ch = sqm[j][:, off:off + TF]
                    sqp = psB.tile([128, TF], f32, tag="sq2")
                    nc.tensor.matmul(sqp, lhsT=meansBD, rhs=pch,
                                     start=True, stop=False)
                    nc.tensor.matmul(sqp, lhsT=ident, rhs=sqm_ch,
                                     start=False, stop=True)
                    # masked entries go ~ -M -> Sqrt -> NaN, killed by max
                    dist = wB.tile([128, TF], f16, tag="dist")
                    nc.scalar.activation(dist, sqp, Act.Sqrt, bias=b_part)
                    h = wB.tile([128, TF], f16, tag="h")
                    if t % 2 == 0:
                        nc.vector.tensor_scalar(h, dist, DELTA_VAR, 0.0,
                                                op0=Alu.subtract, op1=Alu.max)
                    else:
                        nc.gpsimd.tensor_scalar(h, dist, DELTA_VAR, 0.0,
                                                op0=Alu.subtract, op1=Alu.max)
                    h2d = wB.tile([128, TF], f16, tag="h2")
                    nc.vector.tensor_tensor(h2d, h, h, op=Alu.mult)
                    nc.vector.reduce_sum(acc_cols[:, t:t + 1], h2d,
                                         axis=mybir.AxisListType.X)

                var_col = small.tile([128, 1], f32, tag="var_col")
                nc.vector.reduce_sum(var_col, acc_cols,
                                     axis=mybir.AxisListType.X)
                nc.sync.dma_start(out_acc, var_col)

    nc.compile()
    return nc


def _get_nc(reps=1):
    key = ("nc", reps)
    if key not in _CACHE:
        _CACHE[key] = _build(reps)
    return _CACHE[key]


def _host_combine(accs, cnts, means_all, n_objects):
    """Per-core device outputs -> final scalar loss (float64 on host)."""
    losses = []
    for b in range(BS):
        no = float(n_objects[b])
        acc = accs[b].astype(np.float64).reshape(C, 16).sum(axis=0)
        cnt = cnts[b].astype(np.float64).reshape(C, 16).sum(axis=0)
        means = means_all[b].astype(np.float64)            # (i, d)
        valid = (np.arange(NI) < n_objects[b]).astype(np.float64)

        g_sum = np.clip(cnt, 1.0, None)
        var_term = float(np.sum(acc / g_sum) / no)

        means_m = means * valid[:, None]
        diff = means_m[:, None, :] - means_m[None, :, :]
        psq = np.clip((diff * diff).sum(-1), EPS, None)
        pnorm = np.sqrt(psq)
        eye = np.eye(NI)
        margin = 2.0 * DELTA_DIST * (1.0 - eye)
        pair_mask = valid[:, None] * valid[None, :] * (1.0 - eye)
        hinge = np.clip(margin - pnorm, 0.0, None) ** 2 * pair_mask
        denom = max(no * (no - 1.0), 1.0)
        multi = 1.0 if n_objects[b] > 1 else 0.0
        dist_term = float(hinge.sum() / denom * multi)

        mnorm = np.sqrt(np.clip((means_m * means_m).sum(-1), EPS, None)) * valid
        reg_term = float(mnorm.sum() / no)

        losses.append(VAR_W * var_term + DIST_W * dist_term + REG_W * reg_term)
    return np.float32(np.mean(losses))


def _run(prediction, target, n_objects, trace=False, reps=1, **spmd_kwargs):
    from concourse.bass_utils import run_bass_kernel_spmd

    nc = _get_nc(reps)
    consts = _host_consts()

    pred = np.ascontiguousarray(np.asarray(prediction, dtype=np.float32))
    targ = np.ascontiguousarray(np.asarray(target, dtype=np.float32))
    nobj = np.asarray(n_objects)

    in_maps = []
    for b in range(BS):
        m = {"p": pred[b].reshape(16, L), "g": targ[b].reshape(16, L)}
        m.update(consts)
        in_maps.append(m)

    res = run_bass_kernel_spmd(nc, in_maps, list(range(N_CORES)),
                               trace=trace, **spmd_kwargs)
    accs = [res.results[b]["out_acc"] for b in range(BS)]
    cnts = [res.results[b]["out_cnt"] for b in range(BS)]
    means = [res.results[b]["out_means"] for b in range(BS)]
    return _host_combine(accs, cnts, means, nobj), res


def kernel(prediction, target, n_objects):
    loss, _ = _run(prediction, target, n_objects)
    return loss


# revision 3
# speedup vs baseline: 3.2957x; 1.0333x over previous
"""Discriminative loss on 8 Trainium2 NeuronCores — v2.

Strategy (data-parallel over batch: one sample per core):
  Same dataflow as v1 (fold (16, L) -> (128, L/8) fp16, PE-transpose gram
  for cluster means, masked per-pixel distance phase), rebalanced so every
  engine stays under the ~93us HBM stream and phase B's critical path is
  short:

  Phase A (streaming, under the casting-DMA shadow):
    - gpsimd casting DMAs load p,g fp16 (32 MiB f32 read per core).
    - counts via DVE reduce; psq = p*p via DVE (4x fp16).
    - PE transposes p/g chunks into grouped [128,512] f16 PSUM tiles;
      Act evacuates p-groups, gpsimd evacuates g-groups; PE accumulates
      the gram.
    - sqm = fp16(p^2 + M*g - M) via PE (onesbd/mibig matmuls) + DVE
      evacuation, parked then copied into g's SBUF (g dies after this).

  Epilogue: means from gram diag blocks; meansBD = block-diag(-2 mu^T)
  fp16; b_part = m2 (per-partition bias).

  Phase B per (128,512) tile: psum = meansBD@p + ident@sqm (2 accum
  matmuls); dist = Act.Sqrt(psum + b_part) — masked entries go negative
  -> NaN; h = max(dist - 0.5, 0) on DVE/gpsimd KILLS NaN (HW-verified);
  h2 = h*h and per-tile accumulation on DVE.

  Host: combines per-core [acc, cnt, means] in float64 (var term, pairwise
  distance + regularizer from means), mean over batch. No collectives.
"""

import sys
import os
import numpy as np

for _p in ("/opt/trn_rl_repo", "/root/.axon_site/_ro/pypackages"):
    if os.path.isdir(_p) and _p not in sys.path:
        sys.path.insert(0, _p)

BS, ND, H, W, NI = 8, 16, 512, 512, 16
L = H * W                  # 262144 pixels per sample
C = 8                      # fold factor (partition = c*16 + x)
R = L // C                 # 32768 folded free dim
NBT = 16                   # big DMA tiles per tensor
TB = R // NBT              # 2048 cols per big tile
NCH = R // 128             # 256 transpose chunks
NT = 64                    # phase-B compute tiles
TF = R // NT               # 512 cols per compute tile
N_CORES = 8
MBIG = 1024.0              # mask offset, exact in fp16
DELTA_VAR = 0.5
DELTA_DIST = 1.5
VAR_W, DIST_W, REG_W = 1.0, 1.0, 0.001
EPS = 1e-12

_CACHE = {}


def _host_consts():
    f16 = np.float16
    ident = np.eye(128, dtype=f16)
    # onesbd[(c',d),(c,i)] = 1 if c == c'  (p2 broadcast over instances)
    onesbd = np.zeros((128, 128), dtype=f16)
    for c in range(C):
        onesbd[16 * c:16 * c + 16, 16 * c:16 * c + 16] = 1.0
    mibig = (MBIG * np.eye(128)).astype(f16)
    # repmat[k, c*16+i] = (k == i): replicates (16,1) m2 to (128,1) over c
    repmat = np.zeros((16, 128), dtype=np.float32)
    for i in range(16):
        repmat[i, i::16] = 1.0
    ident16 = np.eye(16, dtype=np.float32)
    return {
        "ident": ident,
        "onesbd": onesbd,
        "mibig": mibig,
        "repmat": repmat,
        "ident16": ident16,
    }


def _build(reps=1):
    import concourse.bass as bass
    import concourse.tile as tile
    from concourse import bacc, mybir

    f32 = mybir.dt.float32
    f16 = mybir.dt.float16
    Alu = mybir.AluOpType
    Act = mybir.ActivationFunctionType

    nc = bacc.Bacc("TRN2", target_bir_lowering=False, debug=False,
                   num_devices=N_CORES)

    p_dram = nc.dram_tensor("p", [16, L], f32, kind="ExternalInput").ap()
    g_dram = nc.dram_tensor("g", [16, L], f32, kind="ExternalInput").ap()
    ident_d = nc.dram_tensor("ident", [128, 128], f16, kind="ExternalInput").ap()
    onesbd_d = nc.dram_tensor("onesbd", [128, 128], f16, kind="ExternalInput").ap()
    mibig_d = nc.dram_tensor("mibig", [128, 128], f16, kind="ExternalInput").ap()
    repmat_d = nc.dram_tensor("repmat", [16, 128], f32, kind="ExternalInput").ap()
    ident16_d = nc.dram_tensor("ident16", [16, 16], f32, kind="ExternalInput").ap()

    out_acc = nc.dram_tensor("out_acc", [128], f32, kind="ExternalOutput").ap()
    out_cnt = nc.dram_tensor("out_cnt", [128], f32, kind="ExternalOutput").ap()
    out_means = nc.dram_tensor("out_means", [16, 16], f32, kind="ExternalOutput").ap()

    p_fold = p_dram.rearrange("d (c r) -> d c r", c=C).transpose((1, 0, 2))
    g_fold = g_dram.rearrange("d (c r) -> d c r", c=C).transpose((1, 0, 2))

    GRP = 4                  # transpose chunks per PSUM group
    NGR = (TB // 128) // GRP  # groups per big tile (4)

    with tile.TileContext(nc, num_cores=N_CORES) as tc:
        from contextlib import ExitStack, nullcontext
        with ExitStack() as ctx:
            const_pool = ctx.enter_context(tc.tile_pool(name="const", bufs=1))
            ident = const_pool.tile([128, 128], f16, tag="ident")
            nc.sync.dma_start(ident, ident_d)
            onesbd = const_pool.tile([128, 128], f16, tag="onesbd")
            nc.sync.dma_start(onesbd, onesbd_d)
            mibig = const_pool.tile([128, 128], f16, tag="mibig")
            nc.sync.dma_start(mibig, mibig_d)
            repmat = const_pool.tile([16, 128], f32, tag="repmat")
            nc.sync.dma_start(repmat, repmat_d)
            ident16 = const_pool.tile([16, 16], f32, tag="ident16")
            nc.sync.dma_start(ident16, ident16_d)

            p_pool = ctx.enter_context(tc.tile_pool(name="p16", bufs=NBT))
            g_pool = ctx.enter_context(tc.tile_pool(name="g16", bufs=NBT))
            small = ctx.enter_context(tc.tile_pool(name="small", bufs=1))
            gram_pool = ctx.enter_context(
                tc.tile_pool(name="gram", bufs=1, space="PSUM"))
            psT_p = ctx.enter_context(
                tc.tile_pool(name="psT_p", bufs=2, space="PSUM"))
            psT_g = ctx.enter_context(
                tc.tile_pool(name="psT_g", bufs=2, space="PSUM"))
            psSQ = ctx.enter_context(
                tc.tile_pool(name="psSQ", bufs=2, space="PSUM"))
            psB = psSQ
            aT = ctx.enter_context(tc.tile_pool(name="aT", bufs=6))
            wB = ctx.enter_context(tc.tile_pool(name="wB", bufs=8))

            loop = tc.For_i(0, reps, 1) if reps > 1 else nullcontext()
            with loop:
                # resident fp16 copies of p and g, cast on DMA
                p16 = []
                g16 = []
                for j in range(NBT):
                    pt = p_pool.tile([128, TB], f16, tag=f"p{j}", bufs=1)
                    nc.gpsimd.dma_start(pt, p_fold[:, :, j * TB:(j + 1) * TB])
                    gt = g_pool.tile([128, TB], f16, tag=f"g{j}", bufs=1)
                    nc.gpsimd.dma_start(gt, g_fold[:, :, j * TB:(j + 1) * TB])
                    p16.append(pt)
                    g16.append(gt)

                acc_cols = small.tile([128, NT], f32, tag="acc_cols")
                cntA = small.tile([128, NBT], f32, tag="cntA")

                # ------------- Phase A: gram + counts + sqm -------------
                gram = gram_pool.tile([128, 128], f32, tag="gram")
                sqm = []
                for j in range(NBT):
                    nc.vector.reduce_sum(cntA[:, j:j + 1], g16[j],
                                         axis=mybir.AxisListType.X)
                    # transposes (grouped 4 chunks per PSUM tile) + gram
                    for q in range(NGR):
                        tpg = psT_p.tile([128, GRP * 128], f16, tag="tpg")
                        tgg = psT_g.tile([128, GRP * 128], f16, tag="tgg")
                        for k in range(GRP):
                            off = (q * GRP + k) * 128
                            nc.tensor.transpose(
                                tpg[:, k * 128:(k + 1) * 128],
                                p16[j][:, off:off + 128], ident)
                            nc.tensor.transpose(
                                tgg[:, k * 128:(k + 1) * 128],
                                g16[j][:, off:off + 128], ident)
                        rhs = aT.tile([128, GRP * 128], f16, tag="rhs")
                        nc.scalar.copy(rhs, tpg)
                        gT = aT.tile([128, GRP * 128], f16, tag="gT")
                        nc.scalar.copy(gT, tgg)
                        for k in range(GRP):
                            kk = j * (TB // 128) + q * GRP + k
                            nc.tensor.matmul(
                                gram, lhsT=gT[:, k * 128:(k + 1) * 128],
                                rhs=rhs[:, k * 128:(k + 1) * 128],
                                start=(kk == 0), stop=(kk == NCH - 1))
                    # sqm = fp16(p^2 + M*g - M), parked then copied into
                    # g16[j]'s SBUF slot (g dies after the mibig matmul)
                    TPB = TB // TF
                    park = []
                    for tt in range(TPB):
                        off = tt * TF
                        pch = p16[j][:, off:off + TF]
                        gch = g16[j][:, off:off + TF]
                        psq = wB.tile([128, TF], f16, tag="psq")
                        nc.vector.tensor_tensor(psq, pch, pch, op=Alu.mult)
                        sq2 = psSQ.tile([128, TF], f32, tag="sq2")
                        nc.tensor.matmul(sq2, lhsT=onesbd, rhs=psq,
                                         start=True, stop=False)
                        nc.tensor.matmul(sq2, lhsT=mibig, rhs=gch,
                                         start=False, stop=True)
                        st = wB.tile([128, TF], f16, tag="sqt")
                        nc.vector.tensor_scalar(st, sq2, -MBIG, None,
                                                op0=Alu.add)
                        park.append(st)
                    sqm_j = g_pool.tile([128, TB], f16, tag=f"g{j}", bufs=1)
                    for tt in range(TPB):
                        nc.vector.tensor_copy(
                            sqm_j[:, tt * TF:(tt + 1) * TF], park[tt])
                    sqm.append(sqm_j)

                # ---------- epilogue: means, m2, weights ----------
                gram_sb = small.tile([128, 128], f32, tag="gram_sb")
                nc.vector.tensor_copy(gram_sb, gram)

                blocks = small.tile([16, C * 16], f32, tag="blocks")
                cnt8 = small.tile([16, C], f32, tag="cnt8")
                cnt_sb = small.tile([128, 1], f32, tag="cnt_sb")
                nc.vector.reduce_sum(cnt_sb, cntA, axis=mybir.AxisListType.X)
                for c in range(C):
                    nc.sync.dma_start(blocks[:, c * 16:(c + 1) * 16],
                                      gram_sb[16 * c:16 * c + 16,
                                              16 * c:16 * c + 16])
                    nc.scalar.dma_start(cnt8[:, c:c + 1],
                                        cnt_sb[16 * c:16 * c + 16, :])

                mnum = small.tile([16, 16], f32, tag="mnum")
                nc.vector.reduce_sum(
                    mnum, blocks.rearrange("i (c d) -> i d c", c=C),
                    axis=mybir.AxisListType.X)
                gsum = small.tile([16, 1], f32, tag="gsum")
                nc.vector.reduce_sum(gsum, cnt8, axis=mybir.AxisListType.X)
                gsum_c = small.tile([16, 1], f32, tag="gsum_c")
                nc.vector.tensor_scalar(gsum_c, gsum, 1.0, None, op0=Alu.max)
                invg = small.tile([16, 1], f32, tag="invg")
                nc.vector.reciprocal(invg, gsum_c)

                means = small.tile([16, 16], f32, tag="means")
                nc.vector.tensor_scalar(means, mnum, invg, None, op0=Alu.mult)
                nc.sync.dma_start(out_means, means)
                nc.sync.dma_start(out_cnt, cnt_sb)

                # m2 replicated per (c,i) partition -> bias b = m2
                msq = small.tile([16, 16], f32, tag="msq")
                nc.vector.tensor_tensor(msq, means, means, op=Alu.mult)
                m2 = small.tile([16, 1], f32, tag="m2")
                nc.vector.reduce_sum(m2, msq, axis=mybir.AxisListType.X)
                m2p = psSQ.tile([128, 1], f32, tag="sq2")
                nc.tensor.matmul(m2p, lhsT=repmat, rhs=m2,
                                 start=True, stop=True)
                b_part = small.tile([128, 1], f32, tag="b_part")
                nc.vector.tensor_copy(b_part, m2p)
                # meansBD[(c,d),(c,i)] = -2 * means[i,d] (fp16 block diag)
                meansT = psSQ.tile([16, 16], f32, tag="sq2")
                nc.tensor.transpose(meansT, means, ident16)
                mT2 = small.tile([16, 16], f16, tag="mT2")
                nc.scalar.mul(mT2, meansT, -2.0)
                meansBD = small.tile([128, 128], f16, tag="meansBD")
                nc.vector.memset(meansBD, 0.0)
                for c in range(C):
                    nc.sync.dma_start(
                        meansBD[16 * c:16 * c + 16, 16 * c:16 * c + 16], mT2)

                # ------------- Phase B: per-pixel distances -------------
                for t in range(NT):
                    j, off = divmod(t * TF, TB)
                    pch = p16[j][:, off:off + TF]
                    sqm_ch = sqm[j][:, off:off + TF]
                    sqp = psB.tile([128, TF], f32, tag="sq2")
                    nc.tensor.matmul(sqp, lhsT=meansBD, rhs=pch,
                                     start=True, stop=False)
                    nc.tensor.matmul(sqp, lhsT=ident, rhs=sqm_ch,
                                     start=False, stop=True)
                    # masked entries go ~ -M -> Sqrt -> NaN, killed by max
                    dist = wB.tile([128, TF], f16, tag="dist")
                    nc.scalar.activation(dist, sqp, Act.Sqrt, bias=b_part)
                    h = wB.tile([128, TF], f16, tag="h")
                    nc.vector.tensor_scalar(h, dist, DELTA_VAR, 0.0,
                                            op0=Alu.subtract, op1=Alu.max)
                    h2d = wB.tile([128, TF], f16, tag="h2")
                    nc.vector.tensor_tensor(h2d, h, h, op=Alu.mult)
                    nc.vector.reduce_sum(acc_cols[:, t:t + 1], h2d,
                                         axis=mybir.AxisListType.X)

                var_col = small.tile([128, 1], f32, tag="var_col")
                nc.vector.reduce_sum(var_col, acc_cols,
                                     axis=mybir.AxisListType.X)
                nc.sync.dma_start(out_acc, var_col)

    nc.compile()
    return nc


def _get_nc(reps=1):
    key = ("nc", reps)
    if key not in _CACHE:
        _CACHE[key] = _build(reps)
    return _CACHE[key]


def _host_combine(accs, cnts, means_all, n_objects):
    """Per-core device outputs -> final scalar loss (float64 on host)."""
    losses = []
    for b in range(BS):
        no = float(n_objects[b])
        acc = accs[b].astype(np.float64).reshape(C, 16).sum(axis=0)
        cnt = cnts[b].astype(np.float64).reshape(C, 16).sum(axis=0)
        means = means_all[b].astype(np.float64)            # (i, d)
        valid = (np.arange(NI) < n_objects[b]).astype(np.float64)

        g_sum = np.clip(cnt, 1.0, None)
        var_term = float(np.sum(acc / g_sum) / no)

        means_m = means * valid[:, None]
        diff = means_m[:, None, :] - means_m[None, :, :]
        psq = np.clip((diff * diff).sum(-1), EPS, None)
        pnorm = np.sqrt(psq)
        eye = np.eye(NI)
        margin = 2.0 * DELTA_DIST * (1.0 - eye)
        pair_mask = valid[:, None] * valid[None, :] * (1.0 - eye)
        hinge = np.clip(margin - pnorm, 0.0, None) ** 2 * pair_mask
        denom = max(no * (no - 1.0), 1.0)
        multi = 1.0 if n_objects[b] > 1 else 0.0
        dist_term = float(hinge.sum() / denom * multi)

        mnorm = np.sqrt(np.clip((means_m * means_m).sum(-1), EPS, None)) * valid
        reg_term = float(mnorm.sum() / no)

        losses.append(VAR_W * var_term + DIST_W * dist_term + REG_W * reg_term)
    return np.float32(np.mean(losses))


def _run(prediction, target, n_objects, trace=False, reps=1, **spmd_kwargs):
    from concourse.bass_utils import run_bass_kernel_spmd

    nc = _get_nc(reps)
    consts = _host_consts()

    pred = np.ascontiguousarray(np.asarray(prediction, dtype=np.float32))
    targ = np.ascontiguousarray(np.asarray(target, dtype=np.float32))
    nobj = np.asarray(n_objects)

    in_maps = []
    for b in range(BS):
        m = {"p": pred[b].reshape(16, L), "g": targ[b].reshape(16, L)}
        m.update(consts)
        in_maps.append(m)

    res = run_bass_kernel_spmd(nc, in_maps, list(range(N_CORES)),
                               trace=trace, **spmd_kwargs)
    accs = [res.results[b]["out_acc"] for b in range(BS)]
    cnts = [res.results[b]["out_cnt"] for b in range(BS)]
    means = [res.results[b]["out_means"] for b in range(BS)]
    return _host_combine(accs, cnts, means, nobj), res


def kernel(prediction, target, n_objects):
    loss, _ = _run(prediction, target, n_objects)
    return loss


# revision 4
# speedup vs baseline: 3.5214x; 1.0685x over previous
"""Discriminative loss on 8 Trainium2 NeuronCores — v2.

Strategy (data-parallel over batch: one sample per core):
  Same dataflow as v1 (fold (16, L) -> (128, L/8) fp16, PE-transpose gram
  for cluster means, masked per-pixel distance phase), rebalanced so every
  engine stays under the ~93us HBM stream and phase B's critical path is
  short:

  Phase A (streaming, under the casting-DMA shadow):
    - gpsimd casting DMAs load p,g fp16 (32 MiB f32 read per core).
    - counts via DVE reduce; psq = p*p via DVE (4x fp16).
    - PE transposes p/g chunks into grouped [128,512] f16 PSUM tiles;
      Act evacuates p-groups, gpsimd evacuates g-groups; PE accumulates
      the gram.
    - sqm = fp16(p^2 + M*g - M) via PE (onesbd/mibig matmuls) + DVE
      evacuation, parked then copied into g's SBUF (g dies after this).

  Epilogue: means from gram diag blocks; meansBD = block-diag(-2 mu^T)
  fp16; b_part = m2 (per-partition bias).

  Phase B per (128,512) tile: psum = meansBD@p + ident@sqm (2 accum
  matmuls); dist = Act.Sqrt(psum + b_part) — masked entries go negative
  -> NaN; h = max(dist - 0.5, 0) on DVE/gpsimd KILLS NaN (HW-verified);
  h2 = h*h and per-tile accumulation on DVE.

  Host: combines per-core [acc, cnt, means] in float64 (var term, pairwise
  distance + regularizer from means), mean over batch. No collectives.
"""

import sys
import os
import numpy as np

for _p in ("/opt/trn_rl_repo", "/root/.axon_site/_ro/pypackages"):
    if os.path.isdir(_p) and _p not in sys.path:
        sys.path.insert(0, _p)

BS, ND, H, W, NI = 8, 16, 512, 512, 16
L = H * W                  # 262144 pixels per sample
C = 8                      # fold factor (partition = c*16 + x)
R = L // C                 # 32768 folded free dim
NBT = 16                   # big DMA tiles per tensor
TB = R // NBT              # 2048 cols per big tile
NCH = R // 128             # 256 transpose chunks
NT = 64                    # phase-B compute tiles
TF = R // NT               # 512 cols per compute tile
N_CORES = 8
MBIG = 1024.0              # mask offset, exact in fp16
DELTA_VAR = 0.5
DELTA_DIST = 1.5
VAR_W, DIST_W, REG_W = 1.0, 1.0, 0.001
EPS = 1e-12

_CACHE = {}


def _host_consts():
    f16 = np.float16
    ident = np.eye(128, dtype=f16)
    # onesbd[(c',d),(c,i)] = 1 if c == c'  (p2 broadcast over instances)
    onesbd = np.zeros((128, 128), dtype=f16)
    for c in range(C):
        onesbd[16 * c:16 * c + 16, 16 * c:16 * c + 16] = 1.0
    mibig = (MBIG * np.eye(128)).astype(f16)
    # repmat[k, c*16+i] = (k == i): replicates (16,1) m2 to (128,1) over c
    repmat = np.zeros((16, 128), dtype=np.float32)
    for i in range(16):
        repmat[i, i::16] = 1.0
    ident16 = np.eye(16, dtype=np.float32)
    return {
        "ident": ident,
        "onesbd": onesbd,
        "mibig": mibig,
        "repmat": repmat,
        "ident16": ident16,
    }


def _build(reps=1):
    import concourse.bass as bass
    import concourse.tile as tile
    from concourse import bacc, mybir

    f32 = mybir.dt.float32
    f16 = mybir.dt.float16
    Alu = mybir.AluOpType
    Act = mybir.ActivationFunctionType

    nc = bacc.Bacc("TRN2", target_bir_lowering=False, debug=False,
                   num_devices=N_CORES)

    p_dram = nc.dram_tensor("p", [16, L], f32, kind="ExternalInput").ap()
    g_dram = nc.dram_tensor("g", [16, L], f32, kind="ExternalInput").ap()
    ident_d = nc.dram_tensor("ident", [128, 128], f16, kind="ExternalInput").ap()
    onesbd_d = nc.dram_tensor("onesbd", [128, 128], f16, kind="ExternalInput").ap()
    mibig_d = nc.dram_tensor("mibig", [128, 128], f16, kind="ExternalInput").ap()
    repmat_d = nc.dram_tensor("repmat", [16, 128], f32, kind="ExternalInput").ap()
    ident16_d = nc.dram_tensor("ident16", [16, 16], f32, kind="ExternalInput").ap()

    out_acc = nc.dram_tensor("out_acc", [128], f32, kind="ExternalOutput").ap()
    out_cnt = nc.dram_tensor("out_cnt", [128], f32, kind="ExternalOutput").ap()
    out_means = nc.dram_tensor("out_means", [16, 16], f32, kind="ExternalOutput").ap()

    p_fold = p_dram.rearrange("d (c r) -> d c r", c=C).transpose((1, 0, 2))
    g_fold = g_dram.rearrange("d (c r) -> d c r", c=C).transpose((1, 0, 2))

    GRP = 4                  # transpose chunks per PSUM group
    NGR = (TB // 128) // GRP  # groups per big tile (4)

    with tile.TileContext(nc, num_cores=N_CORES) as tc:
        from contextlib import ExitStack, nullcontext
        with ExitStack() as ctx:
            const_pool = ctx.enter_context(tc.tile_pool(name="const", bufs=1))
            ident = const_pool.tile([128, 128], f16, tag="ident")
            nc.sync.dma_start(ident, ident_d)
            onesbd = const_pool.tile([128, 128], f16, tag="onesbd")
            nc.sync.dma_start(onesbd, onesbd_d)
            mibig = const_pool.tile([128, 128], f16, tag="mibig")
            nc.sync.dma_start(mibig, mibig_d)
            repmat = const_pool.tile([16, 128], f32, tag="repmat")
            nc.sync.dma_start(repmat, repmat_d)
            ident16 = const_pool.tile([16, 16], f32, tag="ident16")
            nc.sync.dma_start(ident16, ident16_d)

            p_pool = ctx.enter_context(tc.tile_pool(name="p16", bufs=NBT))
            g_pool = ctx.enter_context(tc.tile_pool(name="g16", bufs=NBT))
            small = ctx.enter_context(tc.tile_pool(name="small", bufs=1))
            gram_pool = ctx.enter_context(
                tc.tile_pool(name="gram", bufs=1, space="PSUM"))
            psT_p = ctx.enter_context(
                tc.tile_pool(name="psT_p", bufs=2, space="PSUM"))
            psT_g = ctx.enter_context(
                tc.tile_pool(name="psT_g", bufs=2, space="PSUM"))
            psSQ = ctx.enter_context(
                tc.tile_pool(name="psSQ", bufs=3, space="PSUM"))
            psB = psSQ
            aT = ctx.enter_context(tc.tile_pool(name="aT", bufs=6))
            wB = ctx.enter_context(tc.tile_pool(name="wB", bufs=4))

            loop = tc.For_i(0, reps, 1) if reps > 1 else nullcontext()
            with loop:
                # resident fp16 copies of p and g, cast on DMA
                p16 = []
                g16 = []
                for j in range(NBT):
                    pt = p_pool.tile([128, TB], f16, tag=f"p{j}", bufs=1)
                    nc.gpsimd.dma_start(pt, p_fold[:, :, j * TB:(j + 1) * TB])
                    gt = g_pool.tile([128, TB], f16, tag=f"g{j}", bufs=1)
                    nc.gpsimd.dma_start(gt, g_fold[:, :, j * TB:(j + 1) * TB])
                    p16.append(pt)
                    g16.append(gt)

                acc_cols = small.tile([128, NT], f32, tag="acc_cols")
                nc.vector.memset(acc_cols, 0.0)
                accA_cols = small.tile([128, NT], f32, tag="accA_cols")
                nc.vector.memset(accA_cols, 0.0)
                cntA = small.tile([128, NBT], f32, tag="cntA")

                # ------------- Phase A: gram + counts + sqm -------------
                gram = gram_pool.tile([128, 128], f32, tag="gram")
                sqm = []
                for j in range(NBT):
                    nc.vector.reduce_sum(cntA[:, j:j + 1], g16[j],
                                         axis=mybir.AxisListType.X)
                    # transposes (grouped 4 chunks per PSUM tile) + gram
                    for q in range(NGR):
                        tpg = psT_p.tile([128, GRP * 128], f16, tag="tpg")
                        tgg = psT_g.tile([128, GRP * 128], f16, tag="tgg")
                        for k in range(GRP):
                            off = (q * GRP + k) * 128
                            nc.tensor.transpose(
                                tpg[:, k * 128:(k + 1) * 128],
                                p16[j][:, off:off + 128], ident)
                            nc.tensor.transpose(
                                tgg[:, k * 128:(k + 1) * 128],
                                g16[j][:, off:off + 128], ident)
                        rhs = aT.tile([128, GRP * 128], f16, tag="rhs")
                        nc.scalar.copy(rhs, tpg)
                        gT = aT.tile([128, GRP * 128], f16, tag="gT")
                        nc.scalar.copy(gT, tgg)
                        for k in range(GRP):
                            kk = j * (TB // 128) + q * GRP + k
                            nc.tensor.matmul(
                                gram, lhsT=gT[:, k * 128:(k + 1) * 128],
                                rhs=rhs[:, k * 128:(k + 1) * 128],
                                start=(kk == 0), stop=(kk == NCH - 1))
                    # sqm = fp16(p^2 + M*g - M), parked then copied into
                    # g16[j]'s SBUF slot (g dies after the mibig matmul)
                    TPB = TB // TF
                    park = []
                    for tt in range(TPB):
                        off = tt * TF
                        pch = p16[j][:, off:off + TF]
                        gch = g16[j][:, off:off + TF]
                        psq = wB.tile([128, TF], f16, tag="psq")
                        nc.vector.tensor_tensor(psq, pch, pch, op=Alu.mult)
                        sq2 = psSQ.tile([128, TF], f32, tag="sq2")
                        nc.tensor.matmul(sq2, lhsT=onesbd, rhs=psq,
                                         start=True, stop=False)
                        nc.tensor.matmul(sq2, lhsT=mibig, rhs=gch,
                                         start=False, stop=True)
                        st = wB.tile([128, TF], f16, tag="sqt")
                        nc.vector.tensor_scalar(st, sq2, -MBIG, None,
                                                op0=Alu.add)
                        park.append(st)
                    sqm_j = g_pool.tile([128, TB], f16, tag=f"g{j}", bufs=1)
                    for tt in range(TPB):
                        nc.vector.tensor_copy(
                            sqm_j[:, tt * TF:(tt + 1) * TF], park[tt])
                    sqm.append(sqm_j)

                # ---------- epilogue: means, m2, weights ----------
                gram_sb = small.tile([128, 128], f32, tag="gram_sb")
                nc.vector.tensor_copy(gram_sb, gram)

                blocks = small.tile([16, C * 16], f32, tag="blocks")
                cnt8 = small.tile([16, C], f32, tag="cnt8")
                cnt_sb = small.tile([128, 1], f32, tag="cnt_sb")
                nc.vector.reduce_sum(cnt_sb, cntA, axis=mybir.AxisListType.X)
                for c in range(C):
                    nc.sync.dma_start(blocks[:, c * 16:(c + 1) * 16],
                                      gram_sb[16 * c:16 * c + 16,
                                              16 * c:16 * c + 16])
                    nc.scalar.dma_start(cnt8[:, c:c + 1],
                                        cnt_sb[16 * c:16 * c + 16, :])

                mnum = small.tile([16, 16], f32, tag="mnum")
                nc.vector.reduce_sum(
                    mnum, blocks.rearrange("i (c d) -> i d c", c=C),
                    axis=mybir.AxisListType.X)
                gsum = small.tile([16, 1], f32, tag="gsum")
                nc.vector.reduce_sum(gsum, cnt8, axis=mybir.AxisListType.X)
                gsum_c = small.tile([16, 1], f32, tag="gsum_c")
                nc.vector.tensor_scalar(gsum_c, gsum, 1.0, None, op0=Alu.max)
                invg = small.tile([16, 1], f32, tag="invg")
                nc.vector.reciprocal(invg, gsum_c)

                means = small.tile([16, 16], f32, tag="means")
                nc.vector.tensor_scalar(means, mnum, invg, None, op0=Alu.mult)
                nc.sync.dma_start(out_means, means)
                nc.sync.dma_start(out_cnt, cnt_sb)

                # m2 replicated per (c,i) partition -> bias b = m2
                msq = small.tile([16, 16], f32, tag="msq")
                nc.vector.tensor_tensor(msq, means, means, op=Alu.mult)
                m2 = small.tile([16, 1], f32, tag="m2")
                nc.vector.reduce_sum(m2, msq, axis=mybir.AxisListType.X)
                m2p = psSQ.tile([128, 1], f32, tag="sq2")
                nc.tensor.matmul(m2p, lhsT=repmat, rhs=m2,
                                 start=True, stop=True)
                b_part = small.tile([128, 1], f32, tag="b_part")
                nc.vector.tensor_copy(b_part, m2p)
                # meansBD[(c,d),(c,i)] = -2 * means[i,d] (fp16 block diag)
                meansT = psSQ.tile([16, 16], f32, tag="sq2")
                nc.tensor.transpose(meansT, means, ident16)
                mT2 = small.tile([16, 16], f16, tag="mT2")
                nc.scalar.mul(mT2, meansT, -2.0)
                meansBD = small.tile([128, 128], f16, tag="meansBD")
                nc.vector.memset(meansBD, 0.0)
                for c in range(C):
                    nc.sync.dma_start(
                        meansBD[16 * c:16 * c + 16, 16 * c:16 * c + 16], mT2)

                # ------------- Phase B: per-pixel distances -------------
                # pairs of (128,512) psum tiles; 1024-wide elementwise chain
                for pp in range(NT // 2):
                    ps2 = []
                    for t in (2 * pp, 2 * pp + 1):
                        j, off = divmod(t * TF, TB)
                        pch = p16[j][:, off:off + TF]
                        sqm_ch = sqm[j][:, off:off + TF]
                        sqp = psB.tile([128, TF], f32, tag="sq2")
                        nc.tensor.matmul(sqp, lhsT=meansBD, rhs=pch,
                                         start=True, stop=False)
                        nc.tensor.matmul(sqp, lhsT=ident, rhs=sqm_ch,
                                         start=False, stop=True)
                        ps2.append(sqp)
                    # masked entries go ~ -M -> Sqrt -> NaN, killed by max
                    distP = wB.tile([128, 2 * TF], f16, tag="distP")
                    nc.scalar.activation(distP[:, 0:TF], ps2[0], Act.Sqrt,
                                         bias=b_part)
                    nc.scalar.activation(distP[:, TF:2 * TF], ps2[1],
                                         Act.Sqrt, bias=b_part)
                    h = wB.tile([128, 2 * TF], f16, tag="h")
                    nc.vector.tensor_scalar(h, distP, DELTA_VAR, 0.0,
                                            op0=Alu.subtract, op1=Alu.max)
                    h2d = wB.tile([128, 2 * TF], f16, tag="h2")
                    if pp % 3 == 2:
                        nc.scalar.activation(h2d, h, Act.Square,
                                             accum_out=accA_cols[:, pp:pp + 1])
                    else:
                        nc.vector.tensor_tensor(h2d, h, h, op=Alu.mult)
                        nc.vector.reduce_sum(acc_cols[:, pp:pp + 1], h2d,
                                             axis=mybir.AxisListType.X)

                var_c1 = small.tile([128, 1], f32, tag="var_c1")
                nc.vector.reduce_sum(var_c1, acc_cols,
                                     axis=mybir.AxisListType.X)
                var_c2 = small.tile([128, 1], f32, tag="var_c2")
                nc.vector.reduce_sum(var_c2, accA_cols,
                                     axis=mybir.AxisListType.X)
                var_col = small.tile([128, 1], f32, tag="var_col")
                nc.vector.tensor_tensor(var_col, var_c1, var_c2, op=Alu.add)
                nc.sync.dma_start(out_acc, var_col)

    nc.compile()
    return nc


def _get_nc(reps=1):
    key = ("nc", reps)
    if key not in _CACHE:
        _CACHE[key] = _build(reps)
    return _CACHE[key]


def _host_combine(accs, cnts, means_all, n_objects):
    """Per-core device outputs -> final scalar loss (float64 on host)."""
    losses = []
    for b in range(BS):
        no = float(n_objects[b])
        acc = accs[b].astype(np.float64).reshape(C, 16).sum(axis=0)
        cnt = cnts[b].astype(np.float64).reshape(C, 16).sum(axis=0)
        means = means_all[b].astype(np.float64)            # (i, d)
        valid = (np.arange(NI) < n_objects[b]).astype(np.float64)

        g_sum = np.clip(cnt, 1.0, None)
        var_term = float(np.sum(acc / g_sum) / no)

        means_m = means * valid[:, None]
        diff = means_m[:, None, :] - means_m[None, :, :]
        psq = np.clip((diff * diff).sum(-1), EPS, None)
        pnorm = np.sqrt(psq)
        eye = np.eye(NI)
        margin = 2.0 * DELTA_DIST * (1.0 - eye)
        pair_mask = valid[:, None] * valid[None, :] * (1.0 - eye)
        hinge = np.clip(margin - pnorm, 0.0, None) ** 2 * pair_mask
        denom = max(no * (no - 1.0), 1.0)
        multi = 1.0 if n_objects[b] > 1 else 0.0
        dist_term = float(hinge.sum() / denom * multi)

        mnorm = np.sqrt(np.clip((means_m * means_m).sum(-1), EPS, None)) * valid
        reg_term = float(mnorm.sum() / no)

        losses.append(VAR_W * var_term + DIST_W * dist_term + REG_W * reg_term)
    return np.float32(np.mean(losses))


def _run(prediction, target, n_objects, trace=False, reps=1, **spmd_kwargs):
    from concourse.bass_utils import run_bass_kernel_spmd

    nc = _get_nc(reps)
    consts = _host_consts()

    pred = np.ascontiguousarray(np.asarray(prediction, dtype=np.float32))
    targ = np.ascontiguousarray(np.asarray(target, dtype=np.float32))
    nobj = np.asarray(n_objects)

    in_maps = []
    for b in range(BS):
        m = {"p": pred[b].reshape(16, L), "g": targ[b].reshape(16, L)}
        m.update(consts)
        in_maps.append(m)

    res = run_bass_kernel_spmd(nc, in_maps, list(range(N_CORES)),
                               trace=trace, **spmd_kwargs)
    accs = [res.results[b]["out_acc"] for b in range(BS)]
    cnts = [res.results[b]["out_cnt"] for b in range(BS)]
    means = [res.results[b]["out_means"] for b in range(BS)]
    return _host_combine(accs, cnts, means, nobj), res


def kernel(prediction, target, n_objects):
    loss, _ = _run(prediction, target, n_objects)
    return loss
